# revision 19
# baseline (speedup 1.0000x reference)
"""ColBERT MaxSim loss kernel for Trainium2 (8 NeuronCores).

Strategy: shard the document axis c (512) 8-way -> 64 docs/core.
Host pre-transposes both operands so the contraction dim h lands on
SBUF partitions; the device does matmuls + segmented max-reduce only.
The tiny epilogue (sum over s, /T, logsumexp, mean) runs on host.

Matmul precision: PE upconverts fp16 inputs to FP22 exactly and forms
exact e10m23 products, so fp16 inputs give input-rounding-only error
(~2^-13 rel per element). "float16x3" splits each operand into
hi+lo fp16 parts and accumulates 3 passes in PSUM for ~fp32 accuracy.
"""

import numpy as np

import concourse.bacc as bacc
import concourse.bass as bass
import concourse.tile as tile
from concourse import mybir
from concourse.bass_utils import run_bass_kernel_spmd

N_CORES = 8
B, S, H = 32, 32, 128
C, D = 512, 128
C_LOC = C // N_CORES  # 64 docs per core
T = B * S             # 1024 query tokens
TEMPERATURE = 0.02

N_TCHUNK = T // 128            # 8 chunks of 128 tokens (partition dim)
GROUP_DOCS = 16                # docs per psum group (4 banks)
N_GROUP = C_LOC // GROUP_DOCS  # 4 psum groups per t-chunk
GCOLS = GROUP_DOCS * D         # 2048 columns per group
PSUM_BUFS = 2

# "float16" (1 pass) or "float16x3" (hi/lo split, 3 accumulating passes)
MM_DTYPE = "float16"

LAST_RESULTS = None

_NC_CACHE = {}


def _build(mode: str) -> bass.Bass:
    f16 = mybir.dt.float16
    f32 = mybir.dt.float32
    n_parts = 2 if mode == "float16x3" else 1

    nc = bacc.Bacc(None, target_bir_lowering=False)
    # hi/lo parts stacked on the leading axis
    qT = nc.dram_tensor("qT", [n_parts, H, T], f16, kind="ExternalInput")
    pT = nc.dram_tensor(
        "pT", [N_GROUP, n_parts, H, GCOLS], f16, kind="ExternalInput"
    )
    m_out = nc.dram_tensor("m_out", [T, C_LOC], f32, kind="ExternalOutput")

    with tile.TileContext(nc) as tc:
        with (
            tc.tile_pool(name="consts", bufs=1) as consts,
            tc.tile_pool(name="psum", bufs=PSUM_BUFS, space="PSUM") as psum_pool,
            tc.tile_pool(name="mres", bufs=2) as m_pool,
        ):
            qT_sb = consts.tile([H, n_parts, T], f16)
            nc.sync.dma_start(
                out=qT_sb, in_=qT.rearrange("n h t -> h n t")
            )
            pchunks = []
            for j in range(N_GROUP):
                t = consts.tile([H, n_parts, GCOLS], f16, tag=f"pchunk{j}")
                nc.sync.dma_start(out=t, in_=pT[j].rearrange("n h c -> h n c"))
                pchunks.append(t)

            for k in range(N_TCHUNK):
                m_chunk = m_pool.tile([128, C_LOC], f32)
                q_hi = qT_sb[:, 0, k * 128:(k + 1) * 128]
                for g in range(N_GROUP):
                    ps = psum_pool.tile([128, GCOLS], f32)
                    for i in range(GCOLS // 512):
                        sl = slice(i * 512, (i + 1) * 512)
                        if n_parts == 1:
                            nc.tensor.matmul(
                                ps[:, sl], q_hi, pchunks[g][:, 0, sl],
                                start=True, stop=True,
                            )
                        else:
                            q_lo = qT_sb[:, 1, k * 128:(k + 1) * 128]
                            nc.tensor.matmul(
                                ps[:, sl], q_hi, pchunks[g][:, 0, sl],
                                start=True, stop=False,
                            )
                            nc.tensor.matmul(
                                ps[:, sl], q_hi, pchunks[g][:, 1, sl],
                                start=False, stop=False,
                            )
                            nc.tensor.matmul(
                                ps[:, sl], q_lo, pchunks[g][:, 0, sl],
                                start=False, stop=True,
                            )
                    nc.vector.tensor_reduce(
                        out=m_chunk[:, g * GROUP_DOCS:(g + 1) * GROUP_DOCS],
                        in_=ps.rearrange("p (g d) -> p g d", d=D),
                        axis=mybir.AxisListType.X,
                        op=mybir.AluOpType.max,
                    )
                nc.sync.dma_start(
                    out=m_out[k * 128:(k + 1) * 128, :], in_=m_chunk
                )
    nc.compile()
    return nc


def _get_nc(mode: str) -> bass.Bass:
    if mode not in _NC_CACHE:
        _NC_CACHE[mode] = _build(mode)
    return _NC_CACHE[mode]


def _split_f16(x: np.ndarray, n_parts: int) -> np.ndarray:
    """-> [n_parts, ...] fp16 with x ~= sum(parts)."""
    hi = x.astype(np.float16)
    if n_parts == 1:
        return hi[None]
    lo = (x - hi.astype(np.float32)).astype(np.float16)
    return np.stack([hi, lo])


def kernel(query_embeddings, positive_embeddings):
    global LAST_RESULTS
    q = np.ascontiguousarray(np.asarray(query_embeddings, dtype=np.float32))
    p = np.ascontiguousarray(np.asarray(positive_embeddings, dtype=np.float32))
    assert q.shape == (B, S, H) and p.shape == (C, D, H)
    n_parts = 2 if MM_DTYPE == "float16x3" else 1

    qT = np.ascontiguousarray(q.reshape(T, H).T)          # [H, T]
    qT_parts = _split_f16(qT, n_parts)                    # [n, H, T]
    pT = p.transpose(2, 0, 1)                             # [H, C, D] view
    in_maps = []
    for core in range(N_CORES):
        blk = pT[:, core * C_LOC:(core + 1) * C_LOC, :]   # [H, C_LOC, D]
        # chunk-major: [N_GROUP, H, GCOLS]
        chunks = np.ascontiguousarray(
            blk.reshape(H, N_GROUP, GCOLS).transpose(1, 0, 2)
        )
        p_parts = _split_f16(chunks, n_parts)             # [n, N_GROUP, H, GCOLS]
        in_maps.append({
            "qT": np.ascontiguousarray(qT_parts),
            "pT": np.ascontiguousarray(p_parts.transpose(1, 0, 2, 3)),
        })

    nc = _get_nc(MM_DTYPE)
    res = run_bass_kernel_spmd(
        nc, in_maps, core_ids=list(range(N_CORES)), trace=False
    )
    LAST_RESULTS = res

    m = np.concatenate([r["m_out"] for r in res.results], axis=1)  # [T, C]
    m = m.reshape(B, S, C)
    scores = m.sum(axis=1, dtype=np.float64) / TEMPERATURE         # [B, C]
    mx = scores.max(axis=1, keepdims=True)
    lse = mx[:, 0] + np.log(np.exp(scores - mx).sum(axis=1))
    loss = np.mean(lse - scores[:, 0])
    return np.asarray(loss, dtype=np.float32)


# revision 32
# speedup vs baseline: 1.2051x; 1.2051x over previous
"""ColBERT MaxSim loss kernel for Trainium2 (8 NeuronCores).

Strategy: shard the document axis c (512) 8-way -> 64 docs/core.
Host pre-transposes both operands so the contraction dim h lands on
SBUF partitions; the device does matmuls + segmented max-reduce only.
The tiny epilogue (sum over s, /T, logsumexp, mean) runs on host.

Matmul precision: PE upconverts fp16 inputs to FP22 exactly and forms
exact e10m23 products, so fp16 inputs give input-rounding-only error
(~2^-13 rel per element). "float16x3" splits each operand into
hi+lo fp16 parts and accumulates 3 passes in PSUM for ~fp32 accuracy.
"""

import numpy as np

import concourse.bacc as bacc
import concourse.bass as bass
import concourse.tile as tile
from concourse import mybir
from concourse.bass_utils import run_bass_kernel_spmd

N_CORES = 8
B, S, H = 32, 32, 128
C, D = 512, 128
C_LOC = C // N_CORES  # 64 docs per core
T = B * S             # 1024 query tokens
TEMPERATURE = 0.02

N_TCHUNK = T // 128            # 8 chunks of 128 tokens (partition dim)
GROUP_DOCS = 16                # docs per psum group
SCR_BUFS = 8
M_BUFS = 2

# "float16" (1 pass) or "float16x3" (hi/lo split, 3 accumulating passes)
MM_DTYPE = "float16"

# Psum groups with (index % OFFLOAD_MOD) not in KEEP_RES are offloaded:
# ACT copy-casts PSUM->SBUF fp16, DVE runs a 2x-rate fp16 TT-max tree
# (tensor_tensor max has a 2x_1P uop for 16-bit data; tensor_reduce is
# stuck at 1x). Groups in KEEP_RES use the direct 1x fp32 PSUM reduce.
OFFLOAD_MOD = 3
KEEP_RES = (0,)
TREE_LAG = 2

LAST_RESULTS = None

_NC_CACHE = {}


def _build(mode: str) -> bass.Bass:
    f16 = mybir.dt.float16
    f32 = mybir.dt.float32
    n_parts = 2 if mode == "float16x3" else 1
    N_GROUP = C_LOC // GROUP_DOCS
    GCOLS = GROUP_DOCS * D
    PSUM_BUFS = 8 // (GCOLS // 512)

    nc = bacc.Bacc(None, target_bir_lowering=False)
    # hi/lo parts stacked on the leading axis
    qT = nc.dram_tensor("qT", [n_parts, H, T], f16, kind="ExternalInput")
    pT = nc.dram_tensor(
        "pT", [N_GROUP, n_parts, H, GCOLS], f16, kind="ExternalInput"
    )
    m_out = nc.dram_tensor("m_out", [T, C_LOC], f32, kind="ExternalOutput")

    with tile.TileContext(nc) as tc:
        with (
            tc.tile_pool(name="consts", bufs=1) as consts,
            tc.tile_pool(name="psum", bufs=PSUM_BUFS, space="PSUM") as psum_pool,
            tc.tile_pool(name="mres", bufs=M_BUFS) as m_pool,
            tc.tile_pool(name="scr", bufs=SCR_BUFS) as scr_pool,
        ):
            qT_sb = consts.tile([H, n_parts, T], f16)
            nc.sync.dma_start(
                out=qT_sb, in_=qT.rearrange("n h t -> h n t")
            )
            pchunks = []
            for j in range(N_GROUP):
                t = consts.tile([H, n_parts, GCOLS], f16, tag=f"pchunk{j}")
                nc.sync.dma_start(out=t, in_=pT[j].rearrange("n h c -> h n c"))
                pchunks.append(t)

            pending = []  # deferred DVE tree emitters (one group of lag)
            for k in range(N_TCHUNK):
                m_chunk = m_pool.tile([128, C_LOC], f32)
                q_hi = qT_sb[:, 0, k * 128:(k + 1) * 128]
                for g in range(N_GROUP):
                    ps = psum_pool.tile([128, GCOLS], f32)
                    for i in range(GCOLS // 512):
                        sl = slice(i * 512, (i + 1) * 512)
                        if n_parts == 1:
                            nc.tensor.matmul(
                                ps[:, sl], q_hi, pchunks[g][:, 0, sl],
                                start=True, stop=True,
                            )
                        else:
                            q_lo = qT_sb[:, 1, k * 128:(k + 1) * 128]
                            nc.tensor.matmul(
                                ps[:, sl], q_hi, pchunks[g][:, 0, sl],
                                start=True, stop=False,
                            )
                            nc.tensor.matmul(
                                ps[:, sl], q_hi, pchunks[g][:, 1, sl],
                                start=False, stop=False,
                            )
                            nc.tensor.matmul(
                                ps[:, sl], q_lo, pchunks[g][:, 0, sl],
                                start=False, stop=True,
                            )
                    m_seg = m_chunk[:, g * GROUP_DOCS:(g + 1) * GROUP_DOCS]
                    mx = mybir.AluOpType.max
                    if (k * N_GROUP + g) % OFFLOAD_MOD not in KEEP_RES:
                        # ACT drains PSUM to fp16; DVE 2x TT-max tree runs
                        # one group later (lower priority -> no DVE
                        # head-of-line blocking on the ACT copy).
                        sc = scr_pool.tile([128, GROUP_DOCS, D], f16)
                        nc.scalar.copy(
                            out=sc[:, :, :],
                            in_=ps.rearrange("p (g d) -> p g d", d=D),
                        )

                        def emit_tree(sc=sc, m_seg=m_seg):
                            for w in (64, 32, 16, 8):
                                nc.vector.tensor_tensor(
                                    out=sc[:, :, 0:w], in0=sc[:, :, 0:w],
                                    in1=sc[:, :, w:2 * w], op=mx,
                                )
                            nc.vector.tensor_reduce(
                                out=m_seg,
                                in_=sc[:, :, 0:8],
                                axis=mybir.AxisListType.X,
                                op=mx,
                            )
                        pending.append(emit_tree)
                    else:
                        nc.vector.tensor_reduce(
                            out=m_seg,
                            in_=ps.rearrange("p (g d) -> p g d", d=D),
                            axis=mybir.AxisListType.X,
                            op=mx,
                        )
                    while len(pending) > TREE_LAG:
                        pending.pop(0)()
                while pending:
                    pending.pop(0)()
                nc.sync.dma_start(
                    out=m_out[k * 128:(k + 1) * 128, :], in_=m_chunk
                )
    nc.compile()
    return nc


def _get_nc(mode: str) -> bass.Bass:
    if mode not in _NC_CACHE:
        _NC_CACHE[mode] = _build(mode)
    return _NC_CACHE[mode]


def _split_f16(x: np.ndarray, n_parts: int) -> np.ndarray:
    """-> [n_parts, ...] fp16 with x ~= sum(parts)."""
    hi = x.astype(np.float16)
    if n_parts == 1:
        return hi[None]
    lo = (x - hi.astype(np.float32)).astype(np.float16)
    return np.stack([hi, lo])


def kernel(query_embeddings, positive_embeddings):
    global LAST_RESULTS
    q = np.ascontiguousarray(np.asarray(query_embeddings, dtype=np.float32))
    p = np.ascontiguousarray(np.asarray(positive_embeddings, dtype=np.float32))
    assert q.shape == (B, S, H) and p.shape == (C, D, H)
    n_parts = 2 if MM_DTYPE == "float16x3" else 1
    N_GROUP = C_LOC // GROUP_DOCS
    GCOLS = GROUP_DOCS * D

    qT = np.ascontiguousarray(q.reshape(T, H).T)          # [H, T]
    qT_parts = _split_f16(qT, n_parts)                    # [n, H, T]
    pT = p.transpose(2, 0, 1)                             # [H, C, D] view
    in_maps = []
    for core in range(N_CORES):
        blk = pT[:, core * C_LOC:(core + 1) * C_LOC, :]   # [H, C_LOC, D]
        # chunk-major: [N_GROUP, H, GCOLS]
        chunks = np.ascontiguousarray(
            blk.reshape(H, N_GROUP, GCOLS).transpose(1, 0, 2)
        )
        p_parts = _split_f16(chunks, n_parts)             # [n, N_GROUP, H, GCOLS]
        in_maps.append({
            "qT": np.ascontiguousarray(qT_parts),
            "pT": np.ascontiguousarray(p_parts.transpose(1, 0, 2, 3)),
        })

    nc = _get_nc(MM_DTYPE)
    res = run_bass_kernel_spmd(
        nc, in_maps, core_ids=list(range(N_CORES)), trace=False
    )
    LAST_RESULTS = res

    m = np.concatenate([r["m_out"] for r in res.results], axis=1)  # [T, C]
    m = m.reshape(B, S, C)
    scores = m.sum(axis=1, dtype=np.float64) / TEMPERATURE         # [B, C]
    mx = scores.max(axis=1, keepdims=True)
    lse = mx[:, 0] + np.log(np.exp(scores - mx).sum(axis=1))
    loss = np.mean(lse - scores[:, 0])
    return np.asarray(loss, dtype=np.float32)
